# revision 1
# baseline (speedup 1.0000x reference)
"""Masked L1 loss (anomaly VQ loss) on 8 Trainium2 NeuronCores.

reference math:
    num = sum(|pred - vq[c]| * (1 - mask))   over (N,V,C,T,H,W)
    den = sum(1 - mask) * V*C*T              (mask broadcast over V,C,T)
    out = num / den

Sharding: data-parallel over the batch axis N=8 -> one batch element per core.

Device kernel = streaming |x - vq| row-sums over fp8 pred; no mask on device.
The reference notes the original module substitutes vq := pred.detach() at
masked positions (their L1 term is 0).  We apply the equivalent substitution
during host-side shard prep: masked positions are set to fp8 ZERO, so each
contributes exactly |0 - vq_c| = |vq_c| -- a closed-form constant the host
subtracts back out in f64 (den is computed exactly on host from the int
mask).  The vq bias stays EXACT f32; the only device-side error is unbiased
fp8 rounding noise on pred (~3.5e-4 vs f64; gate 2e-2).

Layout: partitions are (c_lo=8, t=8, h_hi=2) so vq varies per-PARTITION and
only via 3 column groups (c = c_hi*8 + c_lo): the per-c bias becomes a
[128,1] per-partition scalar valid for a whole group, letting compute
instructions span ANY columns of a group.  Free dim per group = (v, h_lo, w)
= 24576 contiguous fp8 bytes per partition.

Work is issued in SPANS of [1,1,2,4,8,8] chunks (chunk = 3072 cols =
1.2us of DMA): small head spans start compute early, big body spans amortize
per-instruction fixed costs (ACT ~0.8us/instr, DVE ~0.2us/instr).  One DMA
per span (3-12KB contiguous rows, ~23B/ns/engine).  Per span the columns
split two ways (measured rates):
  ACT [1664/chunk]: activation(Abs, bias=vq, scale=-1, accum_out=acc col)
      -- self-contained abs + row-sum at ~1.9 col/ns + fixed ~1.1us/span.
  DVE [1408/chunk]: tensor_scalar subtract (fp8->bf16, 2x mode, ~1.9 col/ns)
      then bitwise-AND 0x7FFF in place (u16 4x mode, ~4 col/ns, in 2-chunk
      pieces on big spans so PE starts earlier) = |d|;
      PE ones-matmuls then fold every 512-col block into ONE accumulating
      PSUM region ps[0:32, 0:512] (start on the first matmul of the kernel,
      stop on the last) -- TensorReduce has no 2x/4x modes so DVE must not
      reduce; PE's ldweights tax (~112ns/matmul) rides its idle capacity.
Epilogue: one [0:32,512] reduce folds PSUM into an acc column (x32
replicated rows; host divides), one [128,16] DMA out.  ACT's Abs table and
the PE pipeline are pre-warmed during the DMA head.  Host combines in f64.
"""

import os
import sys

for _p in ("/opt/trn_rl_repo", "/root/.axon_site/_ro/trn_rl_repo"):
    if os.path.isdir(_p) and _p not in sys.path:
        sys.path.insert(0, _p)

import numpy as np

import concourse.bacc as bacc
import concourse.mybir as mybir
import concourse.tile as tile
from concourse.bass_utils import run_bass_kernel_spmd

N_CORES = 8
V, C, T, H, W = 3, 24, 8, 128, 128
P = 128
GROUPS = 3               # c_hi
CHUNKS_PER_GROUP = 8
N_CHUNKS = GROUPS * CHUNKS_PER_GROUP
CHUNK = 3072             # columns per chunk (1 chunk = 1.2us of DMA)
GCOLS = CHUNKS_PER_GROUP * CHUNK   # 24576 columns per group

F32 = mybir.dt.float32
BF16 = mybir.dt.bfloat16
FP8 = mybir.dt.float8e4
U16 = mybir.dt.uint16

ALU = mybir.AluOpType
ACTF = mybir.ActivationFunctionType

# per-chunk column shares (sum = CHUNK); measured rates: DVE sub 1.92 +
# AND 4.12 col/ns (net ~1.18), ACT ~1.2 col/ns, PE keeps pace with DVE
ACT_COLS = 1664
DVE_COLS = 1408
SPANS = (1, 1, 2, 4, 8, 8)   # chunks per span; must tile groups of 8
N_SPANS = len(SPANS)


def build_nc(act_cols=ACT_COLS, dve_cols=DVE_COLS, spans=SPANS,
             warm_act=True, warm_pe=True):
    assert act_cols + dve_cols == CHUNK
    assert sum(spans) == N_CHUNKS
    nc = bacc.Bacc("TRN2", target_bir_lowering=False, debug=False)

    pred_d = nc.declare_dram_parameter("pred", [P, GROUPS * GCOLS], FP8, isOutput=False)
    vqg_d = nc.declare_dram_parameter("vqg_host", [P, GROUPS], F32, isOutput=False)
    out1_d = nc.declare_dram_parameter("out1", [P, 16], F32, isOutput=True)

    n_spans = len(spans)
    max_span = max(spans)
    with tile.TileContext(nc) as tc:
        with (
            tc.tile_pool(name="const", bufs=1) as constp,
            tc.tile_pool(name="predp", bufs=n_spans) as predp,
            tc.tile_pool(name="junkd", bufs=2) as junkdp,
            tc.tile_pool(name="psum", bufs=1, space="PSUM") as psump,
        ):
            vqg = constp.tile([P, GROUPS], F32)
            ones = constp.tile([P, 32], BF16)
            acc = constp.tile([P, 16], F32)
            junk_a = constp.tile([P, max_span * act_cols], FP8)
            ps = psump.tile([P, 512], F32)       # rows 0:32 used
            scratch = psump.tile([P, 512], F32)  # PE warm-up target

            nc.sync.dma_start(vqg[:, :], vqg_d[:, :])
            nc.vector.memset(ones[:, :], 1.0)
            if warm_act:
                nc.scalar.activation(junk_a[:, 0:1], ones[:, 0:1], ACTF.Abs,
                                     bias=0.0, scale=-1.0)
            if warm_pe:
                for _ in range(2):
                    nc.tensor.matmul(scratch[0:32, 0:1], ones[:, :], ones[:, 0:1],
                                     start=True, stop=True)

            # one DMA per span on the sync HWDGE queue, issued up-front (the
            # queue streams them back-to-back; whole shard is SBUF-resident)
            tiles = []
            col0 = 0
            for sp in spans:
                t = predp.tile([P, sp * CHUNK], FP8, tag="pt")
                nc.sync.dma_start(t[:, :], pred_d[:, col0 : col0 + sp * CHUNK])
                tiles.append((t, col0, sp))
                col0 += sp * CHUNK

            # total matmul count, to place start/stop flags
            def blocks_of(sp):
                n = sp * dve_cols
                return (n + 511) // 512

            total_mm = sum(blocks_of(sp) for sp in spans)

            mm_i = 0
            for s, (x, col0, sp) in enumerate(tiles):
                g = col0 // GCOLS
                ya, xd = sp * act_cols, sp * dve_cols

                # ACT: abs + row-sum in one instruction
                nc.scalar.activation(junk_a[:, :ya], x[:, 0:ya], ACTF.Abs,
                                     bias=vqg[:, g : g + 1], scale=-1.0,
                                     accum_out=acc[:, s : s + 1])

                # DVE: subtract then strip the sign bit; on big spans the
                # AND runs in 2-chunk pieces so PE's matmuls start earlier
                jd = junkdp.tile([P, max_span * dve_cols], BF16, tag="jd")
                nc.vector.tensor_scalar(jd[:, :xd], x[:, ya : ya + xd],
                                        vqg[:, g : g + 1], None, op0=ALU.subtract)
                n_and = max(1, sp // 2)
                ac = xd // n_and
                for a in range(n_and):
                    nc.vector.tensor_scalar(
                        jd.bitcast(U16)[:, a * ac : (a + 1) * ac],
                        jd.bitcast(U16)[:, a * ac : (a + 1) * ac],
                        0x7FFF, None, op0=ALU.bitwise_and)

                # PE: accumulate every 512-col block into ps[0:32, ...]
                for b in range(blocks_of(sp)):
                    w = min(512, xd - b * 512)
                    nc.tensor.matmul(ps[0:32, 0:w], ones[:, :],
                                     jd[:, b * 512 : b * 512 + w],
                                     start=(mm_i == 0), stop=(mm_i == total_mm - 1),
                                     skip_group_check=True)
                    mm_i += 1

            # fold PSUM into an acc column (rows 0:32 valid, each total x32)
            nc.vector.tensor_reduce(acc[0:32, n_spans : n_spans + 1], ps[0:32, :],
                                    axis=mybir.AxisListType.X, op=ALU.add)
            nc.sync.dma_start(out1_d[:, :], acc[:, :])

    nc.compile()
    return nc


_NC_CACHE = None


def _get_nc():
    global _NC_CACHE
    if _NC_CACHE is None:
        _NC_CACHE = build_nc()
    return _NC_CACHE


_HOST_STATE = None  # (den, correction) from the last make_in_maps


def make_in_maps(pred, mask_extreme, vq_0):
    import ml_dtypes

    global _HOST_STATE

    fp8 = ml_dtypes.float8_e4m3fn
    p8 = np.ascontiguousarray(pred).astype(fp8)
    mask = np.ascontiguousarray(mask_extreme, dtype=np.int32)
    vqf = np.ascontiguousarray(vq_0, dtype=np.float32)

    # vqg[p, g] = vq[g*8 + (p >> 4)], exact f32
    vq_resh = vqf[0].reshape(GROUPS, 8)           # [c_hi, c_lo]
    vqg = np.ascontiguousarray(vq_resh.T[np.repeat(np.arange(8), 16)])  # [128, 3]

    zero8 = fp8(0.0)
    in_maps = []
    for n in range(N_CORES):
        y = p8[n]  # (V, C, T, H, W)
        y = np.where((mask[n] != 0)[None, None, None], zero8, y)
        # (v, c_hi, c_lo, t, h_hi, h_lo, w) -> (c_lo, t, h_hi, c_hi, v, h_lo, w)
        y = y.reshape(V, GROUPS, 8, T, 2, 64, W).transpose(2, 3, 4, 1, 0, 5, 6)
        x = np.ascontiguousarray(y.reshape(P, GROUPS * GCOLS))
        in_maps.append({"pred": x, "vqg_host": vqg})

    msum = float(mask.sum())
    den = (float(N_CORES * H * W) - msum) * float(V * C * T)
    corr = msum * float(V * T) * float(np.abs(vqf.astype(np.float64)).sum())
    _HOST_STATE = (den, corr)
    return in_maps


def combine(results):
    den, corr = _HOST_STATE
    num = 0.0
    for r in results:
        o1 = np.asarray(r["out1"], dtype=np.float64)  # [128, 16]
        num += o1[:, :N_SPANS].sum()                  # ACT span row-sums
        num += o1[0:32, N_SPANS].sum() / 32.0         # PE fold, x32 rows
    num -= corr
    return np.array(num / den, dtype=np.float32)


def kernel(pred, mask_extreme, vq_0):
    nc = _get_nc()
    in_maps = make_in_maps(pred, mask_extreme, vq_0)
    res = run_bass_kernel_spmd(nc, in_maps, core_ids=list(range(N_CORES)))
    return combine(res.results)


if __name__ == "__main__":
    rng = np.random.default_rng(0)
    pred = rng.standard_normal((8, V, C, T, H, W), dtype=np.float32)
    mask = rng.integers(0, 2, size=(8, H, W)).astype(np.int32)
    vq = rng.standard_normal((1, C), dtype=np.float32)
    got = kernel(pred=pred, mask_extreme=mask, vq_0=vq)
    m = mask.astype(np.float64)[:, None, None, None, :, :]
    w = 1.0 - m
    p64 = pred.astype(np.float64)
    numr = np.abs(p64 - vq.astype(np.float64)[0][None, None, :, None, None, None]) * w
    exp = numr.sum() / (w.sum() * V * C * T)
    print("kernel:", got, "expected:", exp, "rel:", abs(got - exp) / abs(exp))

